# revision 8
# baseline (speedup 1.0000x reference)
"""Multi-head attention on 8 trn2 NeuronCores (B=4, N=2048, C=1024, H=16).

Sharding: data-parallel over batch (4) x tensor-parallel over head-halves
(2); core c handles batch c//2, channels [512*(c%2), 512*(c%2)+512) =
8 heads. Each core computes a partial output projection; the host sums core
pairs and adds the projection bias.

Per-core kernel: one software-pipelined instruction stream.
- All matmul inputs bf16 (fp32 PSUM accumulation); inputs are DMA'd bf16.
- Work units are (q-chunk, head-pair, k-block-pair). Per unit: the two
  heads' scores run as concurrent 64x128 PE row-tiles (even head on SBUF
  partitions 0-63 -> T0, odd on 64-127 -> T8) into the two banks of one
  [128, 2, 512] PSUM tile; one ScalarE exp covers both heads (1024 wide)
  and writes sm directly; DVE applies the mask in place (1024 wide);
  attn@V accumulates [65, 512] per head in PSUM (a ones column in the
  augmented V yields the softmax denominator at partition 64).
- Units are emitted with a one-unit lookahead (deeper during startup) so
  the exp stream never waits on the in-order PE queue.
- QKV projection chunks, per-head normalization (reciprocal + GpSimd
  partition_broadcast + DVE multiply), and the output projection are
  deadline-scheduled fillers pumped between units: they keep the PE dense
  (HAM clock stays at 2.4GHz) and overlap phase 1 and the tail.
- PSUM: score pairs 2x2 banks + attn@V accumulators 2x1 + misc 2x1 = 8.
"""

import os
import sys

for p in ("/opt/trn_rl_repo", "/root/.axon_site/_ro/trn_rl_repo"):
    if os.path.isdir(p) and p not in sys.path:
        sys.path.insert(0, p)

import ml_dtypes
import numpy as np

import concourse.bacc as bacc
import concourse.tile as tile
from concourse import mybir
from concourse.bass_utils import run_bass_kernel_spmd

FP = mybir.dt.float32
FR = mybir.dt.float32r
BF = mybir.dt.bfloat16
EXP = mybir.ActivationFunctionType.Exp

DIM = 1024
NUM_HEADS = 16
HEAD_DIM = 64
SCALE = HEAD_DIM ** -0.5
B, N = 4, 2048
NCORES = 8


def build_attention(n=N, c=DIM, cp=DIM // 2, hd=HEAD_DIM, scale=SCALE):
    hpc = cp // hd          # 8 heads per core
    HP = hpc // 2           # 4 head pairs (== MB blocks)
    CB = c // 128           # 8 contraction blocks
    MB = cp // 128          # 4 c' blocks
    NB = n // 128           # 16 k-token blocks
    QW = 512                # phase-2 q chunk width
    QH = n // QW            # 4 q chunks
    QCB = QW // 128         # 4 token blocks per q chunk
    QC = n // 512           # 4 q-projection column groups (512 wide)
    hd1 = hd + 1

    nc = bacc.Bacc("TRN2", target_bir_lowering=False, debug=False)

    xT = nc.dram_tensor("xT", [c, n], BF, kind="ExternalInput").ap()
    wqT = nc.dram_tensor("wqT", [c, cp], BF, kind="ExternalInput").ap()
    wkT = nc.dram_tensor("wkT", [c, cp], BF, kind="ExternalInput").ap()
    wvT = nc.dram_tensor("wvT", [c, cp], BF, kind="ExternalInput").ap()
    wpT = nc.dram_tensor("wpT", [cp, c], BF, kind="ExternalInput").ap()
    maskT = nc.dram_tensor("maskT", [n, n], BF, kind="ExternalInput").ap()
    out = nc.dram_tensor("out", [n, c], FP, kind="ExternalOutput").ap()

    with tile.TileContext(nc) as tc:
        with (
            tc.tile_pool(name="persist", bufs=1) as pers,
            tc.tile_pool(name="mask", bufs=2) as mpool,
            tc.tile_pool(name="s_m", bufs=6) as smp,
            tc.tile_pool(name="aoT", bufs=2) as aop,
            tc.tile_pool(name="ost", bufs=2) as osp,
            tc.tile_pool(name="dtp", bufs=6) as dtp,
            tc.tile_pool(name="dip", bufs=2) as dip,
            tc.tile_pool(name="bcp", bufs=2) as bcp,
            tc.tile_pool(name="ps_sc", bufs=2, space="PSUM") as psc,
            tc.tile_pool(name="ps_ao", bufs=2, space="PSUM") as pao,
            tc.tile_pool(name="ps_m", bufs=2, space="PSUM") as pmisc,
        ):
            xT_sb = pers.tile([128, CB, n], BF, tag="xT")
            w_sb = {
                "q": pers.tile([128, CB, cp], BF, tag="wq", name="wq_sb"),
                "k": pers.tile([128, CB, cp], BF, tag="wk", name="wk_sb"),
                "v": pers.tile([128, CB, cp], BF, tag="wv", name="wv_sb"),
            }
            qT_sb = pers.tile([128, MB, n], BF, tag="qT")
            kT_sb = pers.tile([128, MB, n], BF, tag="kT")
            vaug_sb = pers.tile([128, NB, hpc * hd1], BF, tag="vaug")
            wp_sb = pers.tile([128, MB, c], BF, tag="wp")

            # ---- input DMAs (sync queue, in consumption order) ----
            w_aps = {"q": wqT, "k": wkT, "v": wvT}
            nc.sync.dma_start(
                w_sb["k"], w_aps["k"].rearrange("(cb p) m -> p cb m", p=128)
            )
            nc.sync.dma_start(
                w_sb["q"], w_aps["q"].rearrange("(cb p) m -> p cb m", p=128)
            )
            for cb in range(CB):
                nc.sync.dma_start(
                    xT_sb[:, cb, :],
                    xT.rearrange("(cb p) n -> p cb n", p=128)[:, cb, :],
                )
            mk_tiles = {}

            def queue_mask(qh):
                mk = mpool.tile([128, NB, QW], BF, tag="mk", name="mk")
                mk_tiles[qh] = mk
                qo = qh * QW
                for kb in range(NB):
                    nc.sync.dma_start(
                        mk[:, kb, :],
                        maskT.rearrange("(kb p) q -> p kb q", p=128)[
                            :, kb, qo:qo + QW
                        ],
                    )

            queue_mask(0)
            nc.sync.dma_start(
                w_sb["v"], w_aps["v"].rearrange("(cb p) m -> p cb m", p=128)
            )
            nc.sync.dma_start(
                wp_sb, wpT.rearrange("(mb p) co -> p mb co", p=128)
            )

            # ones column of vaug (softmax denominator trick)
            vaug4 = vaug_sb.rearrange("p nb (h e) -> p nb h e", e=hd1)
            nc.gpsimd.memset(vaug4[:, :, :, hd:hd1], 1.0)

            # ---- deadline-ordered filler machinery ----
            # each filler emits a small chunk of PE work (~4 matmuls);
            # `due` = qh index by which it should complete.
            import heapq
            emitted = set()
            open_groups = {}
            heap = []
            seq = [0]

            def push(due, fn, marker=None):
                heapq.heappush(heap, (due, seq[0], fn, marker))
                seq[0] += 1

            def pop_one():
                due, _, fn, marker = heapq.heappop(heap)
                fn()
                if marker is not None:
                    emitted.add(marker)

            def pump(quota):
                for _ in range(quota):
                    if not heap:
                        return
                    pop_one()

            def drain_marker(marker):
                while marker not in emitted:
                    assert heap, f"missing {marker}"
                    pop_one()

            def drain_due(max_due):
                while heap and heap[0][0] <= max_due:
                    pop_one()

            def qkv_half(kind, idx, qc, half):
                def fn():
                    cbs = range(4) if half == 0 else range(4, 8)
                    if half == 0:
                        pt = pmisc.tile([128, 512], FP, tag="pm", name="pt")
                        open_groups[(kind, idx, qc)] = pt
                    else:
                        pt = open_groups.pop((kind, idx, qc))
                    if kind == "v":
                        for cb in cbs:
                            nc.tensor.matmul(
                                pt,
                                lhsT=xT_sb[:, cb, idx * 128:(idx + 1) * 128],
                                rhs=w_sb["v"][:, cb, :],
                                start=(cb == 0),
                                stop=(cb == CB - 1),
                            )
                        if half == 1:
                            nc.vector.tensor_copy(
                                vaug4[:, idx, :, 0:hd],
                                pt.rearrange("p (h e) -> p h e", e=hd),
                            )
                    else:
                        dst = qT_sb if kind == "q" else kT_sb
                        for cb in cbs:
                            nc.tensor.matmul(
                                pt,
                                lhsT=w_sb[kind][:, cb, idx * 128:(idx + 1) * 128],
                                rhs=xT_sb[:, cb, qc * 512:(qc + 1) * 512],
                                start=(cb == 0),
                                stop=(cb == CB - 1),
                            )
                        if half == 1:
                            nc.vector.tensor_copy(
                                dst[:, idx, qc * 512:(qc + 1) * 512], pt
                            )
                return fn

            def push_group(due, kind, idx, qc):
                push(due, qkv_half(kind, idx, qc, 0))
                push(due, qkv_half(kind, idx, qc, 1), marker=(kind, idx, qc))

            # deadline-0 work in true consumption order: head pair 0's k/q,
            # then all v blocks, then the later head pairs' k/q
            for qc in range(QC):
                push_group(0, "k", 0, qc)
            push_group(0, "q", 0, 0)
            for nb in range(NB):
                push_group(0, "v", nb, 0)
            for mb in range(1, MB):
                for qc in range(QC):
                    push_group(0, "k", mb, qc)
                push_group(0, "q", mb, 0)
            for qh in range(1, QH):
                for mb in range(MB):
                    push_group(qh - 1, "q", mb, qh)

            def norm_closure(aoT_buf, dtmp, h, hp):
                def fn():
                    dinv = dip.tile([1, QW], FP, tag="dinv", name="dinv")
                    nc.vector.reciprocal_approx_fast(dinv, dtmp)
                    bcs = bcp.tile([128, QW], FP, tag="bcs", name="bcs")
                    nc.gpsimd.partition_broadcast(bcs, dinv)
                    nc.vector.tensor_mul(
                        aoT_buf[h * hd:(h + 1) * hd, hp, :],
                        aoT_buf[h * hd:(h + 1) * hd, hp, :],
                        bcs[h * hd:(h + 1) * hd, :],
                    )
                return fn

            def proj_closure(aoT_buf, qh, nbq, co, ot):
                def fn():
                    pt = pmisc.tile([128, 512], FP, tag="pm", name="pp")
                    for mb in range(MB):
                        nc.tensor.matmul(
                            pt,
                            lhsT=aoT_buf[:, mb, nbq * 128:(nbq + 1) * 128],
                            rhs=wp_sb[:, mb, co * 512:(co + 1) * 512],
                            start=(mb == 0),
                            stop=(mb == MB - 1),
                        )
                    nc.vector.tensor_copy(ot[:, co * 512:(co + 1) * 512], pt)
                    if co == c // 512 - 1:
                        nc.sync.dma_start(
                            out.rearrange("(nb p) co -> p nb co", p=128)[
                                :, qh * QCB + nbq, :
                            ],
                            ot,
                        )
                return fn

            # ---------------- main loop ----------------
            # flattened (qh, hp, kb-pair) units, software-pipelined with a
            # one-unit lookahead: scores/exp/mask of unit u+1 are emitted
            # before attn@V of unit u so the PE never stalls the exp stream.
            UPQ = HP * (NB // 2)        # units per qh chunk
            units = []
            for qh in range(QH):
                for hp in range(HP):
                    for kb in range(0, NB, 2):
                        units.append((qh, hp, kb))

            aoT_bufs = {}
            pao_cur = {}
            sm_hist = {}

            def unit_front(u):
                """Emit scores + exp + mask for unit u."""
                qh, hp, kb = units[u]
                qo = qh * QW
                if u % UPQ == 0:
                    if qh + 1 < QH:
                        queue_mask(qh + 1)
                    if qh >= 2:
                        drain_due(qh - 1)
                    aoT_bufs[qh] = aop.tile([128, MB, QW], BF, tag="aoT",
                                            name="aoT_b")
                if kb == 0:
                    drain_marker(("q", hp, qh))
                drain_marker(("k", hp, kb // 4))
                mk = mk_tiles[qh]
                sm_pair = smp.tile([128, 2, 2, QW], BF, tag="sm", name="sm")
                sm_hist[u] = sm_pair
                for i, kbb in enumerate((kb, kb + 1)):
                    sc = psc.tile([128, 2, QW], FP, tag="sc", name="sc")
                    for h in (0, 1):
                        po = h * hd
                        nc.tensor.matmul(
                            sc[:, h, :],
                            lhsT=kT_sb[po:po + hd, hp,
                                       kbb * 128:(kbb + 1) * 128],
                            rhs=qT_sb[po:po + hd, hp, qo:qo + QW],
                            start=True,
                            stop=True,
                        )
                    nc.scalar.activation(
                        sm_pair[:, i, :, :], sc, EXP, scale=scale,
                    )
                for h in (0, 1):
                    nc.vector.tensor_mul(
                        sm_pair[:, :, h, :],
                        sm_pair[:, :, h, :],
                        mk[:, kb:kb + 2, :],
                    )

            def unit_back(u):
                """Emit attn@V for unit u, plus end-of-pair bookkeeping."""
                qh, hp, kb = units[u]
                if kb == 0:
                    pao_cur[hp % 2] = [
                        pao.tile([hd1, QW], FP, tag="pao", name="pao_e"),
                        pao.tile([hd1, QW], FP, tag="pao", name="pao_o"),
                    ]
                pao_t = pao_cur[hp % 2]
                sm_pair = sm_hist.pop(u)
                for i, kbb in enumerate((kb, kb + 1)):
                    drain_marker(("v", kbb, 0))
                    for h in (0, 1):
                        hg = 2 * hp + h
                        nc.tensor.matmul(
                            pao_t[h],
                            lhsT=vaug_sb[:, kbb, hg * hd1:(hg + 1) * hd1],
                            rhs=sm_pair[:, i, h, :],
                            start=(kbb == 0),
                            stop=(kbb == NB - 1),
                        )
                if kb == NB - 2:
                    aoT_buf = aoT_bufs[qh]
                    for h in (0, 1):
                        nc.vector.tensor_copy(
                            aoT_buf[h * hd:(h + 1) * hd, hp, :],
                            pao_t[h][0:hd, :],
                        )
                        dtmp = dtp.tile([1, QW], FP, tag="dtmp", name="dtmp")
                        nc.vector.tensor_copy(dtmp, pao_t[h][hd:hd1, :])
                        push(min(qh + 1, QH - 1), norm_closure(aoT_buf, dtmp, h, hp))
                    if hp == HP - 1:
                        for nbq in range(QCB):
                            ot = osp.tile([128, c], FP, tag="ot", name="ot")
                            for co in range(c // 512):
                                push(min(qh + 1, QH - 1),
                                     proj_closure(aoT_buf, qh, nbq, co, ot))
                        if qh == QH - 1:
                            mk_tiles.pop(qh)

            NU = len(units)
            front_ptr = [0]

            def emit_fronts(target):
                while front_ptr[0] <= min(target, NU - 1):
                    unit_front(front_ptr[0])
                    front_ptr[0] += 1

            emit_fronts(3)
            for u in range(NU):
                pump(2)
                unit_back(u)
                emit_fronts(u + 4 if u < 8 else u + 2)

            drain_due(QH + 1)
            assert not heap
    nc.compile()
    return nc


def make_in_maps(x, mask, wq, wk, wv, wp):
    bf16 = ml_dtypes.bfloat16
    in_maps = []
    for core in range(NCORES):
        b = core // 2
        g = core % 2
        cs = slice(g * 512, (g + 1) * 512)
        in_maps.append({
            "xT": np.ascontiguousarray(x[b].T).astype(bf16),
            "wqT": np.ascontiguousarray(wq[cs, :].T).astype(bf16),
            "wkT": np.ascontiguousarray(wk[cs, :].T).astype(bf16),
            "wvT": np.ascontiguousarray(wv[cs, :].T).astype(bf16),
            "wpT": np.ascontiguousarray(wp[:, cs].T).astype(bf16),
            "maskT": np.ascontiguousarray(mask[b].T).astype(bf16),
        })
    return in_maps


_NC_CACHE = {}


def _get_nc():
    if "nc" not in _NC_CACHE:
        _NC_CACHE["nc"] = build_attention()
    return _NC_CACHE["nc"]


def kernel(x, mask, wq, wk, wv, wp, bp, _trace=False, _trace_kwargs=None):
    x = np.asarray(x, dtype=np.float32)
    mask = np.asarray(mask)
    wq = np.asarray(wq, dtype=np.float32)
    wk = np.asarray(wk, dtype=np.float32)
    wv = np.asarray(wv, dtype=np.float32)
    wp = np.asarray(wp, dtype=np.float32)
    bp = np.asarray(bp, dtype=np.float32)

    nc = _get_nc()
    in_maps = make_in_maps(x, mask, wq, wk, wv, wp)
    kw = {}
    if _trace:
        kw = {"trace": True, **(_trace_kwargs or {})}
    res = run_bass_kernel_spmd(nc, in_maps, list(range(NCORES)), **kw)
    outs = [np.asarray(r["out"], dtype=np.float32) for r in res.results]
    full = np.empty((B, N, DIM), dtype=np.float32)
    for b in range(B):
        full[b] = outs[2 * b] + outs[2 * b + 1] + bp[None, :]
    if _trace:
        return full, res
    return full


if __name__ == "__main__":
    nc = build_attention()
    print("built ok")
